# revision 41
# baseline (speedup 1.0000x reference)
"""Trainium2 Bass kernel for nn_MultiHeadAttention (B=4,T=2048,C=1024,H=16,D=64).

Sharding: tensor-parallel over heads. 8 cores x 2 heads each.
Per core: QKV column slices (128 dims), full attention for its 2 heads,
Wo row slice -> partial output summed on host.

v4 design (vs v1 baseline at 1.35ms, v3 at 1.05ms):
- ALL matmuls in bf16: fp32r streams at ~2 cycles/row on TRN2 HW (fp32
  peak is half of bf16), so bf16 halves tensor-engine time. PSUM stays
  f32; final partial-output DMA stays f32.
- Attention pipelined by one key-group: S/exp of group g+1 issue before
  PV of g, so the in-order PE queue never waits a full exp latency.
- Software pipelining: phase1 (QKV+RoPE) of batch b+1 and phase3 (Wo) of
  batch b-1 emitted in quanta inside attention(b)'s g-loop.
- x-chunk DMA prefetched one q-chunk ahead (double-buffered xs pool).
- Row-broadcasts via PE outer-products (sel2 block-ones lhsT), no DRAM
  roundtrip broadcasts.
- RoPE add + squares on GpSimd; PSUM->SBUF copies on DVE; exp on ACT.

Layouts per core, per batch b:
  qt/kt [128, T] bf16 : rows = 2 heads x 64 dims, cols = seq.
  vsb   [128, NKT, 130] bf16 : per key-tile [V_h0 | ones | V_h1 | ones].
  S.T   psum [128 keys, 2, 512 q] f32 -> exp (ACT, scale 1/8) -> bf16
  PV    : lhsT = V_aug [128, 65] -> psum [65, 512]: rows 0:64 = O.T,
          row 64 = sumexp.
  Wo    : lhsT = yT [128, 128t] @ WoT [128, 512] -> psum f32 -> out f32.

Measured 679us HW (8-core SPMD, rel err 9.2e-3). Final trace state:
warm mm issue rate 216ns (p25), but HAM half-clock dwell is still ~270us
(K=8 only 57% of span) seeded by 11 PE gaps >3.4us (57us: batch-0
prologue ~35us, batch boundaries ~4us each, end barrier). Next steps,
in expected-value order:
  1. PV reformulation: make stexp the stationary operand (FWL, bf16)
     and stream vsb columns (65/mm) so sumexp lands per-PARTITION;
     normalization becomes tensor_scalar (kills outer-products + rcp
     row plumbing); y emerges [q, d] needing 16 [128,128] transposes
     per batch for the Wo lhsT. ~50-100us if LDWEIGHTS overlaps.
  2. Batch-0 prologue: attention(0) cannot start before p1(0)+apply
     completes (S reads all key tiles) -- shrink by pipelining the 4
     qk chunks 2-wide (pairwise RR is deadlock-safe; 4-wide is not
     with mm512 bufs=2 + xs rotation).
  3. Both failed attempts to move work across engines broke silently:
     GpSimd cannot write partitions it does not read (only DVE reading
     PSUM crosses partitions), and DVE tensor ops corrupt with mixed
     bf16/f32 INPUTS (f32 PSUM in0 + f32 in1 -> bf16 out is fine).
"""
import sys

sys.path.insert(0, "/opt/trn_rl_repo")
import numpy as np
import ml_dtypes

BF16NP = ml_dtypes.bfloat16

N_CORES = 8
B_FULL, T_FULL, C = 4, 2048, 1024
H, D = 16, 64
HPC = H // N_CORES          # heads per core = 2
M2 = HPC * D                # 128
EPS = 1e-6

_NC_CACHE: dict = {}


def build_nc(B: int, T: int):
    import concourse.bass as bass
    import concourse.mybir as mybir
    from concourse import bacc
    from concourse.tile import TileContext

    BF16 = mybir.dt.bfloat16
    F32 = mybir.dt.float32
    AF = mybir.ActivationFunctionType
    ALU = mybir.AluOpType

    TT = B * T
    NCIN = C // 128             # 8 contraction tiles for projections
    CPB = T // 512              # 4 chunks of 512 tokens per batch
    NKT = T // 128              # 16 key tiles per batch
    NQC = T // 512              # 4 q chunks per batch
    NG = NKT // 2               # 8 key groups (KGS=2) per q chunk

    nc = bacc.Bacc("TRN2", target_bir_lowering=False, debug=False,
                   num_devices=N_CORES)

    xT_d = nc.dram_tensor("xT", [C, TT], BF16, kind="ExternalInput")
    wq_d = nc.dram_tensor("wq", [C, M2], BF16, kind="ExternalInput")
    wk_d = nc.dram_tensor("wk", [C, M2], BF16, kind="ExternalInput")
    wv_d = nc.dram_tensor("wv", [C, M2], BF16, kind="ExternalInput")
    wo_d = nc.dram_tensor("wo", [M2, C], BF16, kind="ExternalInput")
    cos_d = nc.dram_tensor("cos2", [M2, T], F32, kind="ExternalInput")
    sin_d = nc.dram_tensor("sin2s", [M2, T], F32, kind="ExternalInput")
    ident_d = nc.dram_tensor("ident", [128, 128], BF16, kind="ExternalInput")
    ones2_d = nc.dram_tensor("ones2c", [128, 2], BF16, kind="ExternalInput")
    sel2_d = nc.dram_tensor("sel2", [2, 128], BF16, kind="ExternalInput")
    ones66_d = nc.dram_tensor("ones66", [128, T // 128, 66], BF16,
                              kind="ExternalInput")
    out_d = nc.dram_tensor("out", [TT, C], F32, kind="ExternalOutput")

    with TileContext(nc) as tc:
        with (
            tc.tile_pool(name="const", bufs=1) as cp,
            tc.tile_pool(name="big", bufs=2) as bigp,
            tc.tile_pool(name="xs", bufs=4) as xsp,
            tc.tile_pool(name="attn", bufs=2) as atp,
            tc.tile_pool(name="scr", bufs=2) as scp,
            tc.tile_pool(name="drs", bufs=2, space="DRAM") as drp,
            tc.tile_pool(name="ps", bufs=1, space="PSUM") as psp,
        ):
            wq_sb = cp.tile([128, NCIN, M2], BF16, tag="wq")
            wk_sb = cp.tile([128, NCIN, M2], BF16, tag="wk")
            wv_sb = cp.tile([128, NCIN, M2], BF16, tag="wv")
            wo_sb = cp.tile([128, C], BF16, tag="wo")
            cos_sb = cp.tile([128, T], F32, tag="cos")
            sin_sb = cp.tile([128, T], F32, tag="sin")
            ident = cp.tile([128, 128], BF16, tag="ident")
            ones2 = cp.tile([128, 2], BF16, tag="ones2")
            sel2 = cp.tile([2, 128], BF16, tag="sel2")

            nc.sync.dma_start(
                out=wq_sb, in_=wq_d.rearrange("(co ci) m -> ci co m", ci=128))
            nc.sync.dma_start(
                out=wk_sb, in_=wk_d.rearrange("(co ci) m -> ci co m", ci=128))
            nc.sync.dma_start(out=cos_sb, in_=cos_d[:, :])
            nc.sync.dma_start(out=sin_sb, in_=sin_d[:, :])
            nc.sync.dma_start(out=ones2, in_=ones2_d[:, :])
            nc.sync.dma_start(
                out=wv_sb, in_=wv_d.rearrange("(co ci) m -> ci co m", ci=128))
            nc.sync.dma_start(out=ident, in_=ident_d[:, :])
            nc.sync.dma_start(out=sel2, in_=sel2_d[:, :])
            nc.sync.dma_start(out=wo_sb, in_=wo_d[:, :])

            ROPE_SLICES = (((0, 32), (32, 64)), ((32, 64), (0, 32)),
                           ((64, 96), (96, 128)), ((96, 128), (64, 96)))

            st: dict = {}
            xq: dict = {}

            def get_state(b):
                if b in st:
                    return st[b]
                qtb = bigp.tile([128, T], BF16, tag="qtb")
                ktb = bigp.tile([128, T], BF16, tag="ktb")
                ytb = bigp.tile([128, T], BF16, tag="ytb")
                vsb = bigp.tile([128, NKT, 130], BF16, tag="vsb")
                ssq = scp.tile([2, T], F32, tag="ssq", bufs=1)
                ssk = scp.tile([2, T], F32, tag="ssk", bufs=1)
                sc_q = scp.tile([2, T], BF16, tag="sc_q", bufs=1)
                sc_k = scp.tile([2, T], BF16, tag="sc_k", bufs=1)
                sums_t = drp.tile([4, T], F32, tag="sums")
                scales_t = drp.tile([4, T], BF16, tag="scales")
                if b < 2:
                    # ones columns persist in the physical buffer; later
                    # batches reuse them (V copies never touch cols 64/129)
                    nc.sync.dma_start(out=vsb[:, :, 64:130],
                                      in_=ones66_d[:, :, :])
                s = dict(qtb=qtb, ktb=ktb, ytb=ytb, vsb=vsb, ssq=ssq,
                         ssk=ssk, sc_q=sc_q, sc_k=sc_k, sums_t=sums_t,
                         scales_t=scales_t)
                st[b] = s
                return s

            def prefetch_x(b, c):
                """Issue the x-chunk DMA ahead of its consuming quantum."""
                if (b, c) in xq or b >= B or c >= CPB:
                    return
                x_sb = xsp.tile([128, NCIN, 512], BF16, tag="x")
                nc.sync.dma_start(
                    out=x_sb,
                    in_=xT_d[:, b * T + c * 512: b * T + (c + 1) * 512]
                    .rearrange("(co ci) t -> ci co t", ci=128))
                xq[(b, c)] = x_sb

            def p1_qk_gen(b, c):
                """Q/K projections + RoPE + sumsq for one 512-token chunk.
                x tile stays in xq for the later V pass."""
                s = get_state(b)
                cc = slice(c * 512, (c + 1) * 512)
                prefetch_x(b, c)
                x_sb = xq[(b, c)]
                yield
                for name, w_sb, dkey, sskey, sgtag in (
                        ("q", wq_sb, "qtb", "ssq", "sgA"),
                        ("k", wk_sb, "ktb", "ssk", "sgB")):
                    dst = s[dkey]
                    ps = psp.tile([128, 512], F32, tag="mm512", bufs=2)
                    for ci in range(NCIN // 2):
                        nc.tensor.matmul(ps, w_sb[:, ci], x_sb[:, ci],
                                         start=(ci == 0), stop=False,
                                         skip_group_check=True)
                    yield
                    for ci in range(NCIN // 2, NCIN):
                        nc.tensor.matmul(ps, w_sb[:, ci], x_sb[:, ci],
                                         start=False, stop=(ci == NCIN - 1),
                                         skip_group_check=True)
                    yield
                    # RoPE: dst = ps*cos + rot_half(ps)*sin_signed
                    dd = dst[:, cc]
                    nc.vector.tensor_mul(out=dd, in0=ps, in1=cos_sb[:, cc])
                    rot = scp.tile([128, 512], BF16, tag="rot")
                    for (d0, d1), (s0, s1) in ROPE_SLICES:
                        nc.vector.tensor_mul(
                            out=rot[d0:d1], in0=ps[s0:s1],
                            in1=sin_sb[d0:d1, cc])
                    nc.gpsimd.tensor_add(out=dd, in0=dd, in1=rot)
                    # sum of squares over d (per head) via ones-matmul
                    sq = scp.tile([128, 512], BF16, tag="sq")
                    nc.gpsimd.tensor_mul(out=sq, in0=dd, in1=dd)
                    ps_ss = psp.tile([2, 512], F32, tag=sgtag, bufs=1)
                    nc.tensor.matmul(ps_ss, ones2, sq, start=True, stop=True,
                                     skip_group_check=True)
                    nc.vector.tensor_copy(out=s[sskey][:, cc], in_=ps_ss)
                    if c == CPB - 1:
                        # kick the sums roundtrip as soon as each row is done
                        nc.sync.dma_start(
                            out=s["sums_t"][(0 if name == "q" else 2):
                                            (2 if name == "q" else 4), :],
                            in_=s[sskey])
                    yield

            def p1_rsqrt_gen(b):
                """rsqrt of mean-square, DVE-only (magic constant + 2 Newton
                steps; no ACT table switch), then scales roundtrip."""
                import concourse.mybir as mybir
                I32 = mybir.dt.int32
                s = get_state(b)
                FP = 4 * T // 128
                pk = scp.tile([128, 5, FP], F32, tag="pk", bufs=1)
                y1b = scp.tile([128, FP], BF16, tag="y1b", bufs=1)
                nc.sync.dma_start(
                    out=pk[:, 0],
                    in_=s["sums_t"][:].rearrange("a t -> (a t)")
                    .rearrange("(p f) -> p f", p=128))
                ms, g, t1, tmp = (pk[:, j] for j in range(1, 5))
                nc.vector.tensor_scalar(out=ms, in0=pk[:, 0], scalar1=1.0 / D,
                                        scalar2=EPS, op0=ALU.mult,
                                        op1=ALU.add)
                # g0 via 0x5f3759df bit trick, then 2 Newton iterations
                nc.vector.tensor_scalar(
                    out=tmp.bitcast(I32), in0=ms.bitcast(I32), scalar1=1,
                    scalar2=0, op0=ALU.logical_shift_right,
                    op1=ALU.bitwise_or)
                nc.vector.tensor_scalar(
                    out=g.bitcast(I32), in0=tmp.bitcast(I32), scalar1=-1,
                    scalar2=0x5F3759DF, op0=ALU.mult, op1=ALU.add)
                for it in range(2):
                    nc.vector.tensor_mul(out=t1, in0=g, in1=g)
                    nc.vector.tensor_mul(out=t1, in0=t1, in1=ms)
                    nc.vector.tensor_scalar(out=t1, in0=t1, scalar1=-0.5,
                                            scalar2=1.5, op0=ALU.mult,
                                            op1=ALU.add)
                    out_ap = g if it == 0 else y1b
                    nc.vector.tensor_mul(out=out_ap, in0=g, in1=t1)
                nc.sync.dma_start(
                    out=s["scales_t"][:].rearrange("a t -> (a t)")
                    .rearrange("(p f) -> p f", p=128),
                    in_=y1b)
                nc.sync.dma_start(out=s["sc_q"][:, :], in_=s["scales_t"][0:2, :])
                nc.sync.dma_start(out=s["sc_k"][:, :], in_=s["scales_t"][2:4, :])
                yield

            def p1_v_gen(b, c):
                """V projection + transpose into vsb for one chunk."""
                s = get_state(b)
                vsb = s["vsb"]
                x_sb = xq.pop((b, c))
                ps = psp.tile([128, 512], F32, tag="mm512", bufs=2)
                for ci in range(NCIN // 2):
                    nc.tensor.matmul(ps, wv_sb[:, ci], x_sb[:, ci],
                                     start=(ci == 0), stop=False,
                                     skip_group_check=True)
                yield
                for ci in range(NCIN // 2, NCIN):
                    nc.tensor.matmul(ps, wv_sb[:, ci], x_sb[:, ci],
                                     start=False, stop=(ci == NCIN - 1),
                                     skip_group_check=True)
                vtmp = scp.tile([128, 512], BF16, tag="vtmp")
                nc.vector.tensor_copy(out=vtmp, in_=ps)
                yield
                for i in range(4):
                    kt_idx = c * 4 + i
                    ps_t = psp.tile([128, 128], BF16,
                                    tag=("sgA" if i % 2 == 0 else "sgB"),
                                    bufs=1)
                    nc.tensor.transpose(ps_t, vtmp[:, i * 128:(i + 1) * 128],
                                        ident)
                    # both head halves in one copy: cols {0:64, 65:129}
                    nc.vector.tensor_copy(
                        out=vsb[:, kt_idx, :]
                        .rearrange("p (h x) -> p h x", h=2)[:, :, 0:64],
                        in_=ps_t[:].rearrange("p (h i) -> p h i", h=2))
                    if i == 1:
                        yield

            def p1_apply_gen(b):
                """Apply norm scales to qtb/ktb via outer-product bcasts."""
                s = get_state(b)
                for c in range(CPB):
                    cc = slice(c * 512, (c + 1) * 512)
                    for dkey, sckey, sgtag in (("qtb", "sc_q", "sgA"),
                                               ("ktb", "sc_k", "sgB")):
                        bps = psp.tile([128, 512], F32, tag=sgtag, bufs=1)
                        nc.tensor.matmul(bps, sel2, s[sckey][:, cc],
                                         start=True, stop=True,
                                         skip_group_check=True)
                        nc.vector.tensor_mul(out=s[dkey][:, cc],
                                             in0=s[dkey][:, cc], in1=bps)
                    yield

            def p1_chain(b):
                return ([p1_qk_gen(b, c) for c in range(CPB)]
                        + [p1_rsqrt_gen(b)]
                        + [p1_v_gen(b, c) for c in range(CPB)]
                        + [p1_apply_gen(b)])

            def p3_chunk_gen(b, qc):
                """Wo projection + partial-output DMA for 4 token tiles."""
                s = get_state(b)
                for tt in range(qc * 4, qc * 4 + 4):
                    for oc in range(C // 512):
                        pso = psp.tile([128, 512], F32, tag="mm512", bufs=2)
                        nc.tensor.matmul(
                            pso, s["ytb"][:, tt * 128:(tt + 1) * 128],
                            wo_sb[:, oc * 512:(oc + 1) * 512],
                            start=True, stop=True, skip_group_check=True)
                        ob = scp.tile([128, 512], F32, tag="ob")
                        if oc == 0:
                            nc.vector.tensor_copy(out=ob, in_=pso)
                        else:
                            nc.scalar.copy(out=ob, in_=pso)
                        nc.sync.dma_start(
                            out=out_d[b * T + tt * 128: b * T + (tt + 1) * 128,
                                      oc * 512:(oc + 1) * 512],
                            in_=ob)
                    yield

            def attn_qc(b, qc, fillers):
                """Attention for one 512-query chunk; KGS=2, heads on
                separate PSUM tags, pipelined by one key-group."""
                s = get_state(b)
                qtb, ktb, vsb, ytb = s["qtb"], s["ktb"], s["vsb"], s["ytb"]
                qq = slice(qc * 512, (qc + 1) * 512)
                ot = [psp.tile([65, 512], F32, tag=f"ot{h}", bufs=1,
                               name=f"ot{h}")
                      for h in range(2)]

                def emit_sg(g):
                    stexps = []
                    for h in range(2):
                        hs = slice(h * 64, (h + 1) * 64)
                        sp = psp.tile([128, 2, 512], F32,
                                      tag=("sgA" if h == 0 else "sgB"),
                                      bufs=1, name=f"sg{h}")
                        for i in range(2):
                            ktg = g * 2 + i
                            nc.tensor.matmul(
                                sp[:, i],
                                ktb[hs, ktg * 128:(ktg + 1) * 128],
                                qtb[hs, qq],
                                start=True, stop=True, skip_group_check=True)
                        stexp = atp.tile([128, 2, 512], BF16,
                                         tag=("seA" if h == 0 else "seB"),
                                         name=f"se{h}")
                        nc.scalar.activation(stexp, sp, AF.Exp, scale=0.125)
                        stexps.append(stexp)
                    return stexps

                def advance_filler():
                    while fillers:
                        chain = fillers[0]
                        try:
                            next(chain[0])
                            if len(fillers) > 1:
                                fillers.append(fillers.pop(0))
                            break
                        except StopIteration:
                            chain.pop(0)
                            if not chain:
                                fillers.pop(0)

                # pipelined by one group: S/exp of g+1 issue before PV of g
                pend = emit_sg(0)
                for g in range(NG):
                    nxt = emit_sg(g + 1) if g + 1 < NG else None
                    for h in range(2):
                        for i in range(2):
                            ktg = g * 2 + i
                            nc.tensor.matmul(
                                ot[h], vsb[:, ktg, h * 65:h * 65 + 65],
                                pend[h][:, i],
                                start=(ktg == 0), stop=(ktg == NKT - 1),
                                skip_group_check=True)
                    pend = nxt
                    advance_filler()
                    advance_filler()
                # 1/sumexp rows -> per-head outer-product -> normalize ytb
                rcp0 = scp.tile([1, 512], BF16, tag="rcp0")
                rcp1 = scp.tile([1, 512], BF16, tag="rcp1")
                sml = scp.tile([1, 2, 512], F32, tag="sml")
                rcpf = scp.tile([1, 2, 512], F32, tag="rcpf")
                nc.vector.tensor_copy(out=sml[:, 0], in_=ot[0][64:65])
                nc.vector.tensor_copy(out=sml[:, 1], in_=ot[1][64:65])
                nc.vector.reciprocal_approx_fast(out=rcpf[:], in_=sml[:])
                nc.vector.tensor_copy(out=rcp0[:], in_=rcpf[:, 0])
                nc.vector.tensor_copy(out=rcp1[:], in_=rcpf[:, 1])
                # move O rows into ytb (h0 aligned; h1 via DMA shift)
                nc.vector.tensor_copy(out=ytb[0:64, qq], in_=ot[0][0:64])
                stg = scp.tile([64, 512], BF16, tag="stg")
                nc.vector.tensor_copy(out=stg, in_=ot[1][0:64])
                nc.sync.dma_start(out=ytb[64:128, qq], in_=stg)
                for h, rcp in ((0, rcp0), (1, rcp1)):
                    bmm = psp.tile([64, 512], F32, tag="mm512", bufs=2,
                                   name=f"bmm{h}")
                    nc.tensor.matmul(bmm, sel2[0:1, 0:64], rcp[:],
                                     start=True, stop=True,
                                     skip_group_check=True)
                    hs = slice(h * 64, (h + 1) * 64)
                    nc.vector.tensor_mul(out=ytb[hs, qq],
                                         in0=ytb[hs, qq], in1=bmm)

            # ---- schedule: software-pipelined batches ----
            for c in range(CPB):
                prefetch_x(0, c)
            ch0 = p1_chain(0)
            for pair in (ch0[0:2], ch0[2:4]):
                live = list(pair)
                while live:
                    for g in list(live):
                        try:
                            next(g)
                        except StopIteration:
                            live.remove(g)
            for g in ch0[4:]:
                for _ in g:
                    pass

            for b in range(B):
                p1c = [p1_chain(b + 1)] if b + 1 < B else None
                for qc in range(NQC):
                    # prefetch next batch's x chunks across early slots
                    if b + 1 < B and qc < CPB:
                        prefetch_x(b + 1, qc)
                    fillers = []
                    if p1c is not None and p1c[0]:
                        fillers.append(p1c[0])
                    if qc >= 1:
                        p3c = [p3_chunk_gen(b, qc - 1)]
                        fillers.append(p3c)
                    else:
                        p3c = None
                    attn_qc(b, qc, fillers)
                    # drain p3 at qc boundary; p1 chain persists across slots
                    if p3c is not None:
                        for f in p3c:
                            for _ in f:
                                pass
                # finish any leftover p1 quanta before next batch's attention
                if p1c is not None:
                    for g in p1c[0]:
                        for _ in g:
                            pass
                for _ in p3_chunk_gen(b, NQC - 1):
                    pass

    nc.compile()
    return nc


def make_core_inputs(x, cos, sin, Wq, Wk, Wv, Wo, B, T):
    """Host-side sharding. Returns list of 8 input dicts."""
    TT = B * T
    xT = np.ascontiguousarray(
        np.asarray(x, np.float32).reshape(TT, C).T.astype(BF16NP))
    cosT = np.asarray(cos, np.float32).reshape(T, D).T      # [64, T]
    sinT = np.asarray(sin, np.float32).reshape(T, D).T
    sin_signed = np.concatenate([-sinT[0:32], sinT[32:64]], axis=0)
    cos2 = np.ascontiguousarray(np.concatenate([cosT, cosT], axis=0))
    sin2 = np.ascontiguousarray(np.concatenate([sin_signed, sin_signed],
                                               axis=0))
    ones2c = np.zeros((128, 2), BF16NP)
    ones2c[0:64, 0] = 1.0
    ones2c[64:128, 1] = 1.0
    sel2 = np.zeros((2, 128), BF16NP)
    sel2[0, 0:64] = 1.0
    sel2[1, 64:128] = 1.0
    in_maps = []
    for core in range(N_CORES):
        rows = slice(core * M2, (core + 1) * M2)
        in_maps.append({
            "xT": xT,
            "wq": np.ascontiguousarray(
                np.asarray(Wq, np.float32)[rows].T.astype(BF16NP)),
            "wk": np.ascontiguousarray(
                np.asarray(Wk, np.float32)[rows].T.astype(BF16NP)),
            "wv": np.ascontiguousarray(
                np.asarray(Wv, np.float32)[rows].T.astype(BF16NP)),
            "wo": np.ascontiguousarray(
                np.asarray(Wo, np.float32)[:, rows].T.astype(BF16NP)),
            "cos2": cos2,
            "sin2s": sin2,
            "ident": np.eye(128, dtype=BF16NP),
            "ones2c": ones2c,
            "sel2": sel2,
            "ones66": np.ones((128, T // 128, 66), BF16NP),
        })
    return in_maps


def kernel(x, cos, sin, Wq, Wk, Wv, Wo):
    from concourse.bass_utils import run_bass_kernel_spmd

    B, T = x.shape[0], x.shape[1]
    key = (B, T)
    if key not in _NC_CACHE:
        _NC_CACHE[key] = build_nc(B, T)
    nc = _NC_CACHE[key]
    in_maps = make_core_inputs(x, cos, sin, Wq, Wk, Wv, Wo, B, T)
    res = run_bass_kernel_spmd(nc, in_maps, core_ids=list(range(N_CORES)))
    out = np.zeros((B * T, C), np.float64)
    for r in res.results:
        out += r["out"].astype(np.float64)
    return out.astype(np.float32).reshape(B, T, C)
